# revision 27
# baseline (speedup 1.0000x reference)
"""Trainium2 Bass kernel for nn_CrossLayer (B=8, C=256, S=2048, D=64).

Reference computation (per batch b):
    scores = b_i @ c_i^T               [S, S]
    mid    = softmax(scores, axis=-1)  row softmax over m
    out    = a_i @ mid^T + a_i         [C, S]

Strategy: data-parallel over batch - one batch per NeuronCore (8 cores).
Everything is computed in the "column" layout scoresT[m, n] so the softmax
contraction axis m sits on SBUF partitions, which is what the second matmul
needs.  The n axis is split into two halves of 1024.

v2 design (vs the f32r baseline):
  * Phase-1 inputs are fp16 (precision measured: 2.9e-3 scale-rel absmax
    vs 2.2e-3 for f32 - fp16's 11-bit mantissa keeps score error ~0.005
    absolute, harmless through exp).  fp16 streams 1 col/cycle vs ~2 for
    f32r: phase-1 matmul time halves.
  * K=64 < 128 wastes half the PE array, so phase-1 m-tiles are processed
    in PAIRS with tile_position row-packing: m-tile A contracts on array
    rows 0-63 (cT/bT copy in partitions 0-63), m-tile B on rows 64-127
    (duplicate copy in partitions 64-127).  The two matmul streams run
    concurrently in different row-groups: ~2x phase-1 throughput.
  * Phase-2 (o2[c,n] += aT[m,c].T @ E[m,n]) is serialized over the two
    c-chunks: ct0 accumulates in-loop (1 pair behind phase-1), ct1 MMs
    trail 2 pairs behind, reading E tiles parked in SBUF.  This keeps o2
    at 2 PSUM banks per live c-chunk so PSUM = sc ring (2x2 banks) + o2
    ring (2x2 banks) exactly.
  * 1/Z: rbZ = allones[128,128].T @ zacc broadcasts the column sum Z to
    all 128 partitions in one matmul, then ACT does Ln + Exp(-x) on
    [128,1024] (ACT cost depends only on free-dim elems/lane, so the
    broadcast is free) -> r_sb in SBUF, no PSUM->SBUF copy, no [1,N] ops.
  * Final normalize runs in bf16 (t1, residual, output): DVE 16-bit rate
    and half the output DMA bytes.  Measured total error 5.7e-3 << 2e-2.

PSUM budget (8 banks): sc ring 2 x [128,1024] = 4, o2 ring 2 x [128,1024]
= 4.  The per-half rbZ tile borrows an sc ring slot.
"""

from contextlib import ExitStack

import numpy as np
import ml_dtypes

import concourse.bass as bass
import concourse.tile as tile
from concourse import mybir
from concourse.vector_clock import ScopedClock, VectorClock
from concourse.bass_utils import run_bass_kernel_spmd

F32 = mybir.dt.float32
FP16 = mybir.dt.float16
BF16 = mybir.dt.bfloat16

B, C, S, D = 8, 256, 2048, 64
N_CORES = 8
MT = S // 128       # 16 m-tiles
W = 1024            # n-window (half) width
NH = S // W         # 2 halves
NCT = C // 128      # 2 c-chunks
NP = MT // 2        # 8 m-tile pairs per half
N_WARM = 5          # PE warmup matmuls (pstate ramp during input DMA)


class PatchedTileContext(tile.TileContext):
    """This walrus build caps sync waits per SP Drain/NoOp at <3; the stock
    TileContext tail drain carries one wait per outstanding semaphore.
    Split them one-per-NOP before a clean drain."""

    def _drain_and_barrier(self, tick_clock, wait_clock):
        gclock = tick_clock.global_clock
        nprocs = len(gclock)
        for proc in range(nprocs):
            tick = gclock[proc]
            if tick <= 0:
                continue
            vec = [0] * nprocs
            vec[proc] = tick
            nop_inst = self.nc.sync.nop(nofuse=True)
            wait_clock.add_sem_waits(
                nop_inst.ins, ScopedClock({None: VectorClock(vec)})
            )
        self.nc.sync.drain()
        self.nc.all_engine_barrier()
        assert self.sems is not None
        popped = self.nc._tile_sem_poison_stack.pop()
        assert popped is self._sem_poison
        self.nc.clear_and_free_semaphores(list(self.sems.allocated().values()))
        self.nc.all_engine_barrier()


def _split_sync_waits_json(raw: bytes, cap: int = 1) -> bytes:
    """This walrus build rejects instructions carrying more than ~1 sync
    wait (setupSyncWait: "Too many sync wait commands").  Rewrite the BIR
    JSON so any instruction keeps at most `cap` waits and the excess move
    to NoOps injected immediately before it in the same engine stream -
    identical semantics, compiler-acceptable encoding."""
    import json

    m = json.loads(raw)
    ctr = 0
    for fn in m["functions"]:
        for bb in fn["blocks"]:
            new_insts = []
            for inst in bb["instructions"]:
                si = inst.get("sync_info") or {}
                ow = si.get("on_wait") or []
                if len(ow) > cap:
                    n_extra = len(ow) - cap
                    for w in ow[:n_extra]:
                        ctr += 1
                        nop = {
                            "engine": inst["engine"],
                            "ins": [],
                            "name": f"I-{90000 + ctr}",
                            "opcode": "NoOp",
                            "outs": [],
                            "sync_info": {"on_update": [], "on_wait": [w]},
                        }
                        if inst.get("debug") is not None:
                            nop["debug"] = inst["debug"]
                        new_insts.append(nop)
                    si["on_wait"] = ow[n_extra:]
                new_insts.append(inst)
            bb["instructions"] = new_insts
    return json.dumps(m).encode()


def build_nc() -> bass.Bass:
    nc = bass.Bass()
    # bTd/cTd[k, :] duplicated: rows 0-63 and 64-127 hold the same [D, S]
    # transposed tensor, so packed matmuls can contract on either array half
    bTd = nc.declare_dram_parameter("bTd", [128, S], FP16, isOutput=False)
    cTd = nc.declare_dram_parameter("cTd", [128, S], FP16, isOutput=False)
    # aTb[k, mt, c] = a[mt*128+k, c] of a^T  (bf16, SBUF-exact layout)
    aTb = nc.declare_dram_parameter("aTb", [128, MT * C], BF16, isOutput=False)
    # ar[p, ct, h, nn] = a[ct*128+p, h*W+nn]  (bf16 residual, SBUF layout)
    ar = nc.declare_dram_parameter("ar", [128, NCT * S], BF16, isOutput=False)
    # outb[p, ct, h, nn] -> out[ct*128+p, h*W+nn]  (bf16)
    outb = nc.declare_dram_parameter("outb", [128, NCT * S], BF16, isOutput=True)

    Exp = mybir.ActivationFunctionType.Exp
    Ln = mybir.ActivationFunctionType.Ln

    with PatchedTileContext(nc) as tc, ExitStack() as ctx:
        # ---------- SBUF pools ----------
        const = ctx.enter_context(tc.tile_pool(name="const", bufs=1))
        inp = ctx.enter_context(tc.tile_pool(name="inp", bufs=1))
        epool = ctx.enter_context(tc.tile_pool(name="epool", bufs=12))
        zpool = ctx.enter_context(tc.tile_pool(name="zpool", bufs=3))
        fin = ctx.enter_context(tc.tile_pool(name="fin", bufs=2))
        outp = ctx.enter_context(tc.tile_pool(name="outp", bufs=4))

        # ---------- PSUM pools (8 banks exactly) ----------
        scp = ctx.enter_context(tc.tile_pool(name="scp", bufs=2, space="PSUM"))
        o2p = ctx.enter_context(tc.tile_pool(name="o2p", bufs=2, space="PSUM"))

        # ---------- constants via memset (no DMA dependency) ----------
        kbias = const.tile([128, 1], F32, tag="kbias")
        nc.vector.memset(kbias, -22.0)
        allones = const.tile([128, 128], BF16, tag="allones")
        nc.vector.memset(allones, 1.0)
        onesc = const.tile([128, 1], BF16, tag="onesc")
        nc.vector.memset(onesc, 1.0)
        warm_sb = const.tile([128, 512], BF16, tag="warm_sb")
        nc.vector.memset(warm_sb, 1.0)

        # dummy activation: forces the exp/ln ACT table load (~1.3us) to
        # happen during the input-DMA window instead of before the first
        # real EXP of the pipeline.
        dummy = const.tile([128, 1], F32, tag="dummy")
        nc.scalar.activation(dummy, kbias, Exp)

        # ---------- input DMAs (phase-1 operands first, split for overlap) --
        cT_sb = inp.tile([128, S], FP16, tag="cT")
        nc.sync.dma_start(out=cT_sb[:, 0:256], in_=cTd[:, 0:256])
        bT_sb = inp.tile([128, S], FP16, tag="bT")
        nc.sync.dma_start(out=bT_sb[:, 0:512], in_=bTd[:, 0:512])
        nc.sync.dma_start(out=bT_sb[:, 512:W], in_=bTd[:, 512:W])
        nc.sync.dma_start(out=cT_sb[:, 256:S], in_=cTd[:, 256:S])
        nc.sync.dma_start(out=bT_sb[:, W:S], in_=bTd[:, W:S])
        aT_sb = inp.tile([128, MT, C], BF16, tag="aT")
        aT_r = aTb.rearrange("p (t c) -> p t c", t=MT)
        nc.sync.dma_start(out=aT_sb[:, 0:4, :], in_=aT_r[:, 0:4, :])
        nc.sync.dma_start(out=aT_sb[:, 4:MT, :], in_=aT_r[:, 4:MT, :])
        ar_sb = inp.tile([128, NCT, NH, W], BF16, tag="ar")
        ar_r = ar.rearrange("p (ct h nn) -> p ct h nn", ct=NCT, h=NH)
        nc.sync.dma_start(out=ar_sb[:, 0, :, :], in_=ar_r[:, 0, :, :])
        nc.sync.dma_start(out=ar_sb[:, 1, :, :], in_=ar_r[:, 1, :, :])
        out_r = outb.rearrange("p (ct h nn) -> p ct h nn", ct=NCT, h=NH)

        # Borrowed third sc buffer: lives in the o2 pool's ring (whose banks
        # are idle until ct1 accumulation starts at pair 3).  Used for the
        # warmup matmuls and then pair 1's A-tile, so the fill-phase EXP
        # chain EXP(p) -> sc slot -> sc(p+1) -> EXP(p+1) is offset by one
        # tile and the warmups never delay the o2 accumulator slots.
        sc_fill = o2p.tile([128, W], F32, name="sc_fill", tag="o2")

        # ---------- PE warmup: ramp pstate while inputs stream in ----------
        def warm_mms(n):
            # dependency-free matmuls: keep the PE HAM activity window busy
            # through pipeline-fill stalls so the clock reaches (and keeps)
            # K=8/8.  Into sc_fill's banks, which carry no real data yet.
            for _ in range(n):
                nc.tensor.matmul(
                    sc_fill[0:1, 0:512],
                    lhsT=onesc[:, 0:1],
                    rhs=warm_sb[:, :],
                    start=True,
                    stop=True,
                    skip_group_check=True,
                )

        warm_mms(N_WARM)

        # ---------- main: two n-halves, software-pipelined ----------
        # Each half's z-reduce / 1/Z / normalize tail is interleaved into the
        # NEXT half's first pair iterations so ACT and PE never drain.  State
        # handed across halves:
        #   pend = dict(h, o2 (accumulators awaiting normalize), E (for the
        #   trailing ct phase-2 drains), zacc14, e15 (z fold-in))
        pend = None

        def sc_pair(h, p, E):
            mtA, mtB = 2 * p, 2 * p + 1
            hw0 = h * W
            if h == 0 and p == 1:
                scA = sc_fill
            else:
                scA = scp.tile([128, W], F32, name=f"scA{h}_{p}", tag="sc")
            scB = scp.tile([128, W], F32, name=f"scB{h}_{p}", tag="sc")
            # packed score matmuls: A contracts on array rows 0-63, B on
            # 64-127 (duplicated operand copies).  Issue order A0,B0,A1,B1:
            # B's LDWEIGHTS targets the other row-group so each B matmul
            # streams concurrently with its A (B dur ~5ns in the trace).
            for j in range(2):
                nc.tensor.matmul(
                    scA[:, j * 512 : (j + 1) * 512],
                    lhsT=cT_sb[0:64, mtA * 128 : (mtA + 1) * 128],
                    rhs=bT_sb[0:64, hw0 + j * 512 : hw0 + (j + 1) * 512],
                    start=True,
                    stop=True,
                    tile_position=(0, 0),
                )
                nc.tensor.matmul(
                    scB[:, j * 512 : (j + 1) * 512],
                    lhsT=cT_sb[64:128, mtB * 128 : (mtB + 1) * 128],
                    rhs=bT_sb[64:128, hw0 + j * 512 : hw0 + (j + 1) * 512],
                    start=True,
                    stop=True,
                    tile_position=(64, 0),
                )
            return scA, scB

        def ct_mms(o2, E, ct, pr):
            # phase-2 matmuls for both m-tiles of pair `pr`, c-chunk ct
            for mt in (2 * pr, 2 * pr + 1):
                for j in range(2):
                    nc.tensor.matmul(
                        o2[ct][:, j * 512 : (j + 1) * 512],
                        lhsT=aT_sb[:, mt, ct * 128 : (ct + 1) * 128],
                        rhs=E[mt][:, j * 512 : (j + 1) * 512],
                        start=(mt == 0),
                        stop=(mt == MT - 1),
                    )

        def rbz_mms(pd, j):
            # Z broadcast: rbZ_j[p, n] = Z[n] for all p via all-ones
            # stationary; the final m-tile's exp (e15) is folded in here as a
            # second accumulating matmul instead of a DVE z-add.
            rbZ = scp.tile([128, 512], F32, name=f"rbZ{pd['h']}_{j}", tag="sc")
            js = slice(j * 512, (j + 1) * 512)
            nc.tensor.matmul(
                rbZ, lhsT=allones[:, :], rhs=pd["zacc14"][:, js],
                start=True, stop=False,
            )
            nc.tensor.matmul(
                rbZ, lhsT=allones[:, :], rhs=pd["e15"][:, js],
                start=False, stop=True,
            )
            return rbZ

        def act_tail(pd):
            # r = exp(-ln(Z)): [128,512] per j chunk, broadcast-ready in SBUF
            h = pd["h"]
            rs = []
            for j in range(2):
                lnt = fin.tile([128, 512], F32, name=f"ln{h}_{j}", tag=f"lnt{j}")
                nc.scalar.activation(lnt, pd["rbZ"][j], Ln)
                rsb = fin.tile([128, 512], F32, name=f"r{h}_{j}", tag=f"rsb{j}")
                nc.scalar.activation(rsb, lnt, Exp, scale=-1.0)
                rs.append(rsb)
            pd["rs"] = rs

        def norm_ct(pd, ct, jorder=(0, 1)):
            # normalize + residual (bf16) and store one c-chunk
            h = pd["h"]
            for j in jorder:
                js = slice(j * 512, (j + 1) * 512)
                t1 = fin.tile([128, 512], BF16, name=f"t1_{h}_{ct}_{j}", tag=f"t1_{j}")
                nc.vector.tensor_mul(t1, pd["o2"][ct][:, js], pd["rs"][j])
                o_sb = outp.tile([128, 512], BF16, name=f"o_{h}_{ct}_{j}", tag=f"o_{j}")
                nc.vector.tensor_add(o_sb, t1, ar_sb[:, ct, h, js])
                nc.sync.dma_start(out=out_r[:, ct, h, js], in_=o_sb)

        for h in range(NH):
            o2 = [
                o2p.tile([128, W], F32, name=f"o2_{h}_{ct}", tag="o2")
                for ct in range(NCT)
            ]
            E = {}
            zacc = None

            for p in range(NP):
                mtA, mtB = 2 * p, 2 * p + 1
                if p == 1 and pend is not None:
                    # previous half: ACT reciprocal chain + last ct1 drain
                    act_tail(pend)
                    ct_mms(pend["o2"], pend["E"], 1, NP - 1)
                scA, scB = sc_pair(h, p, E)
                # phase-2 fillers keep the PE busy while ACT exps this pair;
                # ct1 lags 3 pairs so its o2 bank (freed late by the previous
                # half's normalize) is never on the PE critical path.
                if p == 0 and pend is not None:
                    # previous half: trailing phase-2 drains + Z broadcast,
                    # interleaved so each rbZ lands right when its sc-ring
                    # slot frees (after this half's pair-0 EXPs).
                    ct_mms(pend["o2"], pend["E"], 0, NP - 1)
                    pend["rbZ"] = [rbz_mms(pend, 0)]
                    ct_mms(pend["o2"], pend["E"], 1, NP - 3)
                    pend["rbZ"].append(rbz_mms(pend, 1))
                    ct_mms(pend["o2"], pend["E"], 1, NP - 2)
                if p >= 1:
                    ct_mms(o2, E, 0, p - 1)
                if p >= 3:
                    ct_mms(o2, E, 1, p - 3)
                if h == 0 and p == 0:
                    # fill-phase EXP-wait gap: keep HAM warm
                    warm_mms(3)
                # exp(sc - K) -> bf16 SBUF.  K=-22 keeps Z inside the ACT Ln
                # table's accurate range; softmax shift-invariance cancels it.
                eA = epool.tile([128, W], BF16, name=f"e{h}_{mtA}", tag="e")
                nc.scalar.activation(eA, scA[:, :], Exp, bias=kbias[:, 0:1])
                eB = epool.tile([128, W], BF16, name=f"e{h}_{mtB}", tag="e")
                nc.scalar.activation(eB, scB[:, :], Exp, bias=kbias[:, 0:1])
                E[mtA], E[mtB] = eA, eB
                if p == 1 and pend is not None:
                    # previous half: normalize ct0 (DVE) as soon as r lands
                    norm_ct(pend, 0)
                if p == 2 and pend is not None:
                    norm_ct(pend, 1)
                    pend = None
                # Z partials on DVE (bf16 ping-pong); the last m-tile (m15)
                # is folded into the rbZ matmuls instead.
                for mt, e in ((mtA, eA), (mtB, eB)):
                    if mt == MT - 1:
                        continue
                    znew = zpool.tile([128, W], BF16, name=f"z{h}_{mt}", tag="zacc")
                    if mt == 0:
                        nc.vector.tensor_scalar_mul(znew, e, 1.0)
                    else:
                        nc.vector.tensor_add(znew, zacc, e)
                    zacc = znew

            pend = {
                "h": h,
                "o2": o2,
                "E": E,
                "zacc14": zacc,
                "e15": E[MT - 1],
            }

        # ---------- final half tail (no next half to interleave into) ------
        # rbZ goes early in the PE queue (right after enough drain work to
        # cover the e15 wait) so the ACT Ln/Exp chain starts ASAP; the ct1
        # drains only gate the ct1 normalize, which comes last anyway.
        ct_mms(pend["o2"], pend["E"], 0, NP - 1)
        with tc.high_priority():
            pend["rbZ"] = [rbz_mms(pend, 0), rbz_mms(pend, 1)]
            act_tail(pend)
        ct_mms(pend["o2"], pend["E"], 1, NP - 3)
        ct_mms(pend["o2"], pend["E"], 1, NP - 2)
        norm_ct(pend, 0, jorder=(0,))
        ct_mms(pend["o2"], pend["E"], 1, NP - 1)
        norm_ct(pend, 1, jorder=(0,))
        norm_ct(pend, 0, jorder=(1,))
        norm_ct(pend, 1, jorder=(1,))

    orig_to_json_bytes = nc.to_json_bytes

    def to_json_bytes():
        return _split_sync_waits_json(orig_to_json_bytes())

    nc.to_json_bytes = to_json_bytes
    return nc


_NC_CACHE = None


def _get_nc():
    global _NC_CACHE
    if _NC_CACHE is None:
        _NC_CACHE = build_nc()
    return _NC_CACHE


def kernel(a, b, c, **run_kwargs):
    """a: [8, 256, 2048] f32, b: [8, 2048, 64] f32, c: [8, 2048, 64] f32
    -> [8, 256, 2048] f32"""
    a = np.asarray(a, dtype=np.float32)
    b = np.asarray(b, dtype=np.float32)
    c = np.asarray(c, dtype=np.float32)
    in_maps = []
    for i in range(N_CORES):
        bT = np.ascontiguousarray(b[i].T)  # [D, S]
        cT = np.ascontiguousarray(c[i].T)
        bTd = np.concatenate([bT, bT], axis=0).astype(np.float16)
        cTd = np.concatenate([cT, cT], axis=0).astype(np.float16)
        aT = np.ascontiguousarray(a[i].T)  # [S, C]
        aTb = (
            aT.reshape(MT, 128, C)
            .transpose(1, 0, 2)
            .reshape(128, MT * C)
            .astype(ml_dtypes.bfloat16)
        )
        ar = (
            a[i]
            .reshape(NCT, 128, NH, W)
            .transpose(1, 0, 2, 3)
            .reshape(128, NCT * S)
            .astype(ml_dtypes.bfloat16)
        )
        in_maps.append({"bTd": bTd, "cTd": cTd, "aTb": aTb, "ar": ar})
    res = run_bass_kernel_spmd(_get_nc(), in_maps, list(range(N_CORES)), **run_kwargs)
    out = np.stack(
        [
            np.asarray(res.results[i]["outb"])
            .astype(np.float32)
            .reshape(128, NCT, NH, W)
            .transpose(1, 0, 2, 3)
            .reshape(C, S)
            for i in range(N_CORES)
        ]
    )
    if run_kwargs:
        kernel.last_result = res
    return out


# revision 28
# speedup vs baseline: 1.0271x; 1.0271x over previous
"""Trainium2 Bass kernel for nn_CrossLayer (B=8, C=256, S=2048, D=64).

Reference computation (per batch b):
    scores = b_i @ c_i^T               [S, S]
    mid    = softmax(scores, axis=-1)  row softmax over m
    out    = a_i @ mid^T + a_i         [C, S]

Strategy: data-parallel over batch - one batch per NeuronCore (8 cores).
Everything is computed in the "column" layout scoresT[m, n] so the softmax
contraction axis m sits on SBUF partitions, which is what the second matmul
needs.  The n axis is split into two halves of 1024.

v2 design (vs the f32r baseline):
  * Phase-1 inputs are fp16 (precision measured: 2.9e-3 scale-rel absmax
    vs 2.2e-3 for f32 - fp16's 11-bit mantissa keeps score error ~0.005
    absolute, harmless through exp).  fp16 streams 1 col/cycle vs ~2 for
    f32r: phase-1 matmul time halves.
  * K=64 < 128 wastes half the PE array, so phase-1 m-tiles are processed
    in PAIRS with tile_position row-packing: m-tile A contracts on array
    rows 0-63 (cT/bT copy in partitions 0-63), m-tile B on rows 64-127
    (duplicate copy in partitions 64-127).  The two matmul streams run
    concurrently in different row-groups: ~2x phase-1 throughput.
  * Phase-2 (o2[c,n] += aT[m,c].T @ E[m,n]) is serialized over the two
    c-chunks: ct0 accumulates in-loop (1 pair behind phase-1), ct1 MMs
    trail 2 pairs behind, reading E tiles parked in SBUF.  This keeps o2
    at 2 PSUM banks per live c-chunk so PSUM = sc ring (2x2 banks) + o2
    ring (2x2 banks) exactly.
  * 1/Z: rbZ = allones[128,128].T @ zacc broadcasts the column sum Z to
    all 128 partitions in one matmul, then ACT does Ln + Exp(-x) on
    [128,1024] (ACT cost depends only on free-dim elems/lane, so the
    broadcast is free) -> r_sb in SBUF, no PSUM->SBUF copy, no [1,N] ops.
  * Final normalize runs in bf16 (t1, residual, output): DVE 16-bit rate
    and half the output DMA bytes.  Measured total error 5.7e-3 << 2e-2.

PSUM budget (8 banks): sc ring 2 x [128,1024] = 4, o2 ring 2 x [128,1024]
= 4.  The per-half rbZ tile borrows an sc ring slot.
"""

from contextlib import ExitStack

import numpy as np
import ml_dtypes

import concourse.bass as bass
import concourse.tile as tile
from concourse import mybir
from concourse.vector_clock import ScopedClock, VectorClock
from concourse.bass_utils import run_bass_kernel_spmd

F32 = mybir.dt.float32
FP16 = mybir.dt.float16
BF16 = mybir.dt.bfloat16

B, C, S, D = 8, 256, 2048, 64
N_CORES = 8
MT = S // 128       # 16 m-tiles
W = 1024            # n-window (half) width
NH = S // W         # 2 halves
NCT = C // 128      # 2 c-chunks
NP = MT // 2        # 8 m-tile pairs per half
N_WARM = 5          # PE warmup matmuls (pstate ramp during input DMA)


class PatchedTileContext(tile.TileContext):
    """This walrus build caps sync waits per SP Drain/NoOp at <3; the stock
    TileContext tail drain carries one wait per outstanding semaphore.
    Split them one-per-NOP before a clean drain."""

    def _drain_and_barrier(self, tick_clock, wait_clock):
        gclock = tick_clock.global_clock
        nprocs = len(gclock)
        for proc in range(nprocs):
            tick = gclock[proc]
            if tick <= 0:
                continue
            vec = [0] * nprocs
            vec[proc] = tick
            nop_inst = self.nc.sync.nop(nofuse=True)
            wait_clock.add_sem_waits(
                nop_inst.ins, ScopedClock({None: VectorClock(vec)})
            )
        self.nc.sync.drain()
        self.nc.all_engine_barrier()
        assert self.sems is not None
        popped = self.nc._tile_sem_poison_stack.pop()
        assert popped is self._sem_poison
        self.nc.clear_and_free_semaphores(list(self.sems.allocated().values()))
        self.nc.all_engine_barrier()


def _split_sync_waits_json(raw: bytes, cap: int = 1) -> bytes:
    """This walrus build rejects instructions carrying more than ~1 sync
    wait (setupSyncWait: "Too many sync wait commands").  Rewrite the BIR
    JSON so any instruction keeps at most `cap` waits and the excess move
    to NoOps injected immediately before it in the same engine stream -
    identical semantics, compiler-acceptable encoding."""
    import json

    m = json.loads(raw)
    ctr = 0
    for fn in m["functions"]:
        for bb in fn["blocks"]:
            new_insts = []
            for inst in bb["instructions"]:
                si = inst.get("sync_info") or {}
                ow = si.get("on_wait") or []
                if len(ow) > cap:
                    n_extra = len(ow) - cap
                    for w in ow[:n_extra]:
                        ctr += 1
                        nop = {
                            "engine": inst["engine"],
                            "ins": [],
                            "name": f"I-{90000 + ctr}",
                            "opcode": "NoOp",
                            "outs": [],
                            "sync_info": {"on_update": [], "on_wait": [w]},
                        }
                        if inst.get("debug") is not None:
                            nop["debug"] = inst["debug"]
                        new_insts.append(nop)
                    si["on_wait"] = ow[n_extra:]
                new_insts.append(inst)
            bb["instructions"] = new_insts
    return json.dumps(m).encode()


def build_nc() -> bass.Bass:
    nc = bass.Bass()
    # bTd/cTd[k, :] duplicated: rows 0-63 and 64-127 hold the same [D, S]
    # transposed tensor, so packed matmuls can contract on either array half
    bTd = nc.declare_dram_parameter("bTd", [128, S], FP16, isOutput=False)
    cTd = nc.declare_dram_parameter("cTd", [128, S], FP16, isOutput=False)
    # aTb[k, mt, c] = a[mt*128+k, c] of a^T  (bf16, SBUF-exact layout)
    aTb = nc.declare_dram_parameter("aTb", [128, MT * C], BF16, isOutput=False)
    # ar[p, ct, h, nn] = a[ct*128+p, h*W+nn]  (bf16 residual, SBUF layout)
    ar = nc.declare_dram_parameter("ar", [128, NCT * S], BF16, isOutput=False)
    # outb[p, ct, h, nn] -> out[ct*128+p, h*W+nn]  (bf16)
    outb = nc.declare_dram_parameter("outb", [128, NCT * S], BF16, isOutput=True)

    Exp = mybir.ActivationFunctionType.Exp
    Ln = mybir.ActivationFunctionType.Ln

    with PatchedTileContext(nc) as tc, ExitStack() as ctx:
        # ---------- SBUF pools ----------
        const = ctx.enter_context(tc.tile_pool(name="const", bufs=1))
        inp = ctx.enter_context(tc.tile_pool(name="inp", bufs=1))
        epool = ctx.enter_context(tc.tile_pool(name="epool", bufs=12))
        zpool = ctx.enter_context(tc.tile_pool(name="zpool", bufs=2))
        fin = ctx.enter_context(tc.tile_pool(name="fin", bufs=2))
        outp = ctx.enter_context(tc.tile_pool(name="outp", bufs=4))

        # ---------- PSUM pools (8 banks exactly) ----------
        scp = ctx.enter_context(tc.tile_pool(name="scp", bufs=2, space="PSUM"))
        o2p = ctx.enter_context(tc.tile_pool(name="o2p", bufs=2, space="PSUM"))

        # ---------- constants via memset (no DMA dependency) ----------
        kbias = const.tile([128, 1], F32, tag="kbias")
        nc.vector.memset(kbias, -22.0)
        allones = const.tile([128, 128], BF16, tag="allones")
        nc.vector.memset(allones, 1.0)
        onesc = const.tile([128, 1], BF16, tag="onesc")
        nc.vector.memset(onesc, 1.0)
        warm_sb = const.tile([128, 512], BF16, tag="warm_sb")
        nc.vector.memset(warm_sb, 1.0)

        # dummy activation: forces the exp/ln ACT table load (~1.3us) to
        # happen during the input-DMA window instead of before the first
        # real EXP of the pipeline.
        dummy = const.tile([128, 1], F32, tag="dummy")
        nc.scalar.activation(dummy, kbias, Exp)

        # ---------- input DMAs (phase-1 operands first, split for overlap) --
        cT_sb = inp.tile([128, S], FP16, tag="cT")
        nc.sync.dma_start(out=cT_sb[:, 0:256], in_=cTd[:, 0:256])
        bT_sb = inp.tile([128, S], FP16, tag="bT")
        nc.sync.dma_start(out=bT_sb[:, 0:512], in_=bTd[:, 0:512])
        nc.sync.dma_start(out=bT_sb[:, 512:W], in_=bTd[:, 512:W])
        nc.sync.dma_start(out=cT_sb[:, 256:S], in_=cTd[:, 256:S])
        nc.sync.dma_start(out=bT_sb[:, W:S], in_=bTd[:, W:S])
        aT_sb = inp.tile([128, MT, C], BF16, tag="aT")
        aT_r = aTb.rearrange("p (t c) -> p t c", t=MT)
        nc.sync.dma_start(out=aT_sb[:, 0:4, :], in_=aT_r[:, 0:4, :])
        nc.sync.dma_start(out=aT_sb[:, 4:MT, :], in_=aT_r[:, 4:MT, :])
        ar_sb = inp.tile([128, NCT, NH, W], BF16, tag="ar")
        ar_r = ar.rearrange("p (ct h nn) -> p ct h nn", ct=NCT, h=NH)
        nc.sync.dma_start(out=ar_sb[:, 0, :, :], in_=ar_r[:, 0, :, :])
        nc.sync.dma_start(out=ar_sb[:, 1, :, :], in_=ar_r[:, 1, :, :])
        out_r = outb.rearrange("p (ct h nn) -> p ct h nn", ct=NCT, h=NH)

        # Borrowed third sc buffer: lives in the o2 pool's ring (whose banks
        # are idle until ct1 accumulation starts at pair 3).  Used for the
        # warmup matmuls and then pair 1's A-tile, so the fill-phase EXP
        # chain EXP(p) -> sc slot -> sc(p+1) -> EXP(p+1) is offset by one
        # tile and the warmups never delay the o2 accumulator slots.
        sc_fill = o2p.tile([128, W], F32, name="sc_fill", tag="o2")

        # ---------- PE warmup: ramp pstate while inputs stream in ----------
        def warm_mms(n):
            # dependency-free matmuls: keep the PE HAM activity window busy
            # through pipeline-fill stalls so the clock reaches (and keeps)
            # K=8/8.  Into sc_fill's banks, which carry no real data yet.
            for _ in range(n):
                nc.tensor.matmul(
                    sc_fill[0:1, 0:512],
                    lhsT=onesc[:, 0:1],
                    rhs=warm_sb[:, :],
                    start=True,
                    stop=True,
                    skip_group_check=True,
                )

        warm_mms(N_WARM)

        # ---------- main: two n-halves, software-pipelined ----------
        # Each half's z-reduce / 1/Z / normalize tail is interleaved into the
        # NEXT half's first pair iterations so ACT and PE never drain.  State
        # handed across halves:
        #   pend = dict(h, o2 (accumulators awaiting normalize), E (for the
        #   trailing ct phase-2 drains), zacc14, e15 (z fold-in))
        pend = None

        def sc_pair(h, p, E):
            mtA, mtB = 2 * p, 2 * p + 1
            hw0 = h * W
            if h == 0 and p == 1:
                scA = sc_fill
            else:
                scA = scp.tile([128, W], F32, name=f"scA{h}_{p}", tag="sc")
            scB = scp.tile([128, W], F32, name=f"scB{h}_{p}", tag="sc")
            # packed score matmuls: A contracts on array rows 0-63, B on
            # 64-127 (duplicated operand copies).  Issue order A0,B0,A1,B1:
            # B's LDWEIGHTS targets the other row-group so each B matmul
            # streams concurrently with its A (B dur ~5ns in the trace).
            for j in range(2):
                nc.tensor.matmul(
                    scA[:, j * 512 : (j + 1) * 512],
                    lhsT=cT_sb[0:64, mtA * 128 : (mtA + 1) * 128],
                    rhs=bT_sb[0:64, hw0 + j * 512 : hw0 + (j + 1) * 512],
                    start=True,
                    stop=True,
                    tile_position=(0, 0),
                )
                nc.tensor.matmul(
                    scB[:, j * 512 : (j + 1) * 512],
                    lhsT=cT_sb[64:128, mtB * 128 : (mtB + 1) * 128],
                    rhs=bT_sb[64:128, hw0 + j * 512 : hw0 + (j + 1) * 512],
                    start=True,
                    stop=True,
                    tile_position=(64, 0),
                )
            return scA, scB

        def ct_mms(o2, E, ct, pr):
            # phase-2 matmuls for both m-tiles of pair `pr`, c-chunk ct
            for mt in (2 * pr, 2 * pr + 1):
                for j in range(2):
                    nc.tensor.matmul(
                        o2[ct][:, j * 512 : (j + 1) * 512],
                        lhsT=aT_sb[:, mt, ct * 128 : (ct + 1) * 128],
                        rhs=E[mt][:, j * 512 : (j + 1) * 512],
                        start=(mt == 0),
                        stop=(mt == MT - 1),
                    )

        def rbz_mms(pd, j):
            # Z broadcast: rbZ_j[p, n] = Z[n] for all p via all-ones
            # stationary; the final m-tile's exp (e15) is folded in here as a
            # second accumulating matmul instead of a DVE z-add.
            rbZ = scp.tile([128, 512], F32, name=f"rbZ{pd['h']}_{j}", tag="sc")
            js = slice(j * 512, (j + 1) * 512)
            nc.tensor.matmul(
                rbZ, lhsT=allones[:, :], rhs=pd["zacc14"][:, js],
                start=True, stop=False,
            )
            nc.tensor.matmul(
                rbZ, lhsT=allones[:, :], rhs=pd["e15"][:, js],
                start=False, stop=True,
            )
            return rbZ

        def act_tail(pd):
            # r = exp(-ln(Z)): [128,512] per j chunk, broadcast-ready in SBUF
            h = pd["h"]
            rs = []
            for j in range(2):
                lnt = fin.tile([128, 512], F32, name=f"ln{h}_{j}", tag=f"lnt{j}")
                nc.scalar.activation(lnt, pd["rbZ"][j], Ln)
                rsb = fin.tile([128, 512], F32, name=f"r{h}_{j}", tag=f"rsb{j}")
                nc.scalar.activation(rsb, lnt, Exp, scale=-1.0)
                rs.append(rsb)
            pd["rs"] = rs

        def norm_ct(pd, ct, jorder=(0, 1)):
            # normalize + residual (bf16) and store one c-chunk
            h = pd["h"]
            for j in jorder:
                js = slice(j * 512, (j + 1) * 512)
                t1 = fin.tile([128, 512], BF16, name=f"t1_{h}_{ct}_{j}", tag=f"t1_{j}")
                nc.vector.tensor_mul(t1, pd["o2"][ct][:, js], pd["rs"][j])
                o_sb = outp.tile([128, 512], BF16, name=f"o_{h}_{ct}_{j}", tag=f"o_{j}")
                nc.vector.tensor_add(o_sb, t1, ar_sb[:, ct, h, js])
                nc.sync.dma_start(out=out_r[:, ct, h, js], in_=o_sb)

        for h in range(NH):
            o2 = [
                o2p.tile([128, W], F32, name=f"o2_{h}_{ct}", tag="o2")
                for ct in range(NCT)
            ]
            E = {}
            zacc = None

            for p in range(NP):
                mtA, mtB = 2 * p, 2 * p + 1
                if p == 1 and pend is not None:
                    # previous half: ACT reciprocal chain + last ct1 drain
                    act_tail(pend)
                    ct_mms(pend["o2"], pend["E"], 1, NP - 1)
                scA, scB = sc_pair(h, p, E)
                # phase-2 fillers keep the PE busy while ACT exps this pair;
                # ct1 lags 3 pairs so its o2 bank (freed late by the previous
                # half's normalize) is never on the PE critical path.
                if p == 0 and pend is not None:
                    # previous half: trailing phase-2 drains + Z broadcast,
                    # interleaved so each rbZ lands right when its sc-ring
                    # slot frees (after this half's pair-0 EXPs).
                    ct_mms(pend["o2"], pend["E"], 0, NP - 1)
                    pend["rbZ"] = [rbz_mms(pend, 0)]
                    ct_mms(pend["o2"], pend["E"], 1, NP - 3)
                    pend["rbZ"].append(rbz_mms(pend, 1))
                    ct_mms(pend["o2"], pend["E"], 1, NP - 2)
                if p >= 1:
                    ct_mms(o2, E, 0, p - 1)
                if p >= 3:
                    ct_mms(o2, E, 1, p - 3)
                if h == 0 and p == 0:
                    # fill-phase EXP-wait gap: keep HAM warm
                    warm_mms(3)
                # exp(sc - K) -> bf16 SBUF.  K=-22 keeps Z inside the ACT Ln
                # table's accurate range; softmax shift-invariance cancels it.
                eA = epool.tile([128, W], BF16, name=f"e{h}_{mtA}", tag="e")
                nc.scalar.activation(eA, scA[:, :], Exp, bias=kbias[:, 0:1])
                eB = epool.tile([128, W], BF16, name=f"e{h}_{mtB}", tag="e")
                nc.scalar.activation(eB, scB[:, :], Exp, bias=kbias[:, 0:1])
                E[mtA], E[mtB] = eA, eB
                if p == 1 and pend is not None:
                    # previous half: normalize ct0 (DVE) as soon as r lands
                    norm_ct(pend, 0)
                if p == 2 and pend is not None:
                    norm_ct(pend, 1)
                    pend = None
                # Z partials on DVE (bf16 ping-pong); the last m-tile (m15)
                # is folded into the rbZ matmuls instead.
                for mt, e in ((mtA, eA), (mtB, eB)):
                    if mt == MT - 1:
                        continue
                    znew = zpool.tile([128, W], BF16, name=f"z{h}_{mt}", tag="zacc")
                    if mt == 0:
                        nc.vector.tensor_scalar_mul(znew, e, 1.0)
                    else:
                        nc.vector.tensor_add(znew, zacc, e)
                    zacc = znew

            pend = {
                "h": h,
                "o2": o2,
                "E": E,
                "zacc14": zacc,
                "e15": E[MT - 1],
            }

        # ---------- final half tail (no next half to interleave into) ------
        # rbZ goes early in the PE queue (right after enough drain work to
        # cover the e15 wait) so the ACT Ln/Exp chain starts ASAP; the ct1
        # drains only gate the ct1 normalize, which comes last anyway.
        ct_mms(pend["o2"], pend["E"], 0, NP - 1)
        with tc.high_priority():
            pend["rbZ"] = [rbz_mms(pend, 0), rbz_mms(pend, 1)]
            act_tail(pend)
        ct_mms(pend["o2"], pend["E"], 1, NP - 3)
        ct_mms(pend["o2"], pend["E"], 1, NP - 2)
        norm_ct(pend, 0, jorder=(0,))
        ct_mms(pend["o2"], pend["E"], 1, NP - 1)
        norm_ct(pend, 1, jorder=(0,))
        norm_ct(pend, 0, jorder=(1,))
        norm_ct(pend, 1, jorder=(1,))

    orig_to_json_bytes = nc.to_json_bytes

    def to_json_bytes():
        return _split_sync_waits_json(orig_to_json_bytes())

    nc.to_json_bytes = to_json_bytes
    return nc


_NC_CACHE = None


def _get_nc():
    global _NC_CACHE
    if _NC_CACHE is None:
        _NC_CACHE = build_nc()
    return _NC_CACHE


def kernel(a, b, c, **run_kwargs):
    """a: [8, 256, 2048] f32, b: [8, 2048, 64] f32, c: [8, 2048, 64] f32
    -> [8, 256, 2048] f32"""
    a = np.asarray(a, dtype=np.float32)
    b = np.asarray(b, dtype=np.float32)
    c = np.asarray(c, dtype=np.float32)
    in_maps = []
    for i in range(N_CORES):
        bT = np.ascontiguousarray(b[i].T)  # [D, S]
        cT = np.ascontiguousarray(c[i].T)
        bTd = np.concatenate([bT, bT], axis=0).astype(np.float16)
        cTd = np.concatenate([cT, cT], axis=0).astype(np.float16)
        aT = np.ascontiguousarray(a[i].T)  # [S, C]
        aTb = (
            aT.reshape(MT, 128, C)
            .transpose(1, 0, 2)
            .reshape(128, MT * C)
            .astype(ml_dtypes.bfloat16)
        )
        ar = (
            a[i]
            .reshape(NCT, 128, NH, W)
            .transpose(1, 0, 2, 3)
            .reshape(128, NCT * S)
            .astype(ml_dtypes.bfloat16)
        )
        in_maps.append({"bTd": bTd, "cTd": cTd, "aTb": aTb, "ar": ar})
    res = run_bass_kernel_spmd(_get_nc(), in_maps, list(range(N_CORES)), **run_kwargs)
    out = np.stack(
        [
            np.asarray(res.results[i]["outb"])
            .astype(np.float32)
            .reshape(128, NCT, NH, W)
            .transpose(1, 0, 2, 3)
            .reshape(C, S)
            for i in range(N_CORES)
        ]
    )
    if run_kwargs:
        kernel.last_result = res
    return out
